# revision 3
# baseline (speedup 1.0000x reference)
"""Trainium2 Bass kernel for nn_CortexBlock_59940563583556.

Math note (exact, not an approximation): the reference initializes the
fast-weight state U0 = V0 = 0 inside reference() itself, and every term
of the scan's update to U/V is proportional to ku = k_t^T @ U (zero when
U == 0).  By induction U_t == V_t == 0 for the whole scan, for ANY input
values.  Hence k_fast == 0, score_fast == 0, and (since mix_logit is
added to both logits, softmax is shift-invariant) the block reduces
exactly to:

    q = h @ Wq.T ; k = h @ Wk.T ; v = h @ Wv.T          (per-head split)
    g[b,t,h]  = sigmoid( sum_d q[b,t,h,d] * k[b,t,h,d] / sqrt(64) )
    out       = (g * v  per head) @ Wo.T

m_gate / alpha_scale / Wa / ba / mix_logit do not affect the output.

Sharding: data-parallel over the 8192 rows of the flattened [B*T, D]
activations across 8 NeuronCores (1024 rows each); weights replicated.

Precision: q/k GEMMs in fp8-e4m3 DoubleRow (Wq/Wk host-prescaled x64,
factor divided out in the sigmoid scale); v/out GEMMs bf16.  Same
numerics as the 1.29e-2-rel-err baseline.

Schedule (v2, single fused pass — the measured win over the two-pass
baseline is the 13us post-matmul tail):
  - Only Sync+Scalar have HWDGE queues; gpsimd is SWDGE (slow ~10us
    start, used for the tiles-4-7 halves of h that aren't needed until
    ~25us).  ALL input triggers are emitted at the head of both HWDGE
    engine streams so no trigger is head-of-line blocked behind a
    semaphore-waiting activation (the baseline lost ~2us to this).
  - warmup matmuls on garbage-initialized SBUF start at ~5.9us (gated
    only on a 100ns gpsimd memset) to run the HAM clock ramp + its
    half-clock repayment window during the DMA-bound phase.
  - phase A: q tiles 0-3, kt2-outer / jo-outer (paced by wq+h8 arrival,
    zig-zagged across the two HWDGE queues in consumption order).
  - phase B: k rounds r0-r1 across tiles 0-3 (needs only wk[0:4]), then
    per-tile r2-r3 so k_t completes early and its score chain (sp-mul /
    reduce / sigmoid, split DVE+ACT) frees tile t's PSUM pair before
    v_t needs it.
  - phase C: v tiles 0-3, chain + yT DMA-transpose per tile.
  - phase D (t=4..7): q_t,k_t (fp8 rounds), v_t, then out-GEMM of tile
    t-4 — out work interleaved into the stream instead of a second
    pass; out chunks are copied PSUM->SBUF split ACT/DVE and DMA'd
    immediately on BOTH HWDGE queues.
  - phase E: out-GEMMs 4..7.  The last tile runs chunk-serial (jo0's 8
    kt matmuls, then jo1's) with quarter-width copies so everything but
    the final 128KB piece overlaps matmuls; post-matmul tail ~1us.
PSUM: 8 banks as 4 pairs; in D/E the roles (q,k,v,o) are pinned to
pairs 0..3 so every WAR wait lands >=7us after the prior reader.
"""

import numpy as np
import ml_dtypes

import concourse.bass as bass
import concourse.mybir as mybir
import concourse.tile as tile
from concourse import bacc
from concourse.bass_utils import run_bass_kernel_spmd

F32 = mybir.dt.float32
BF16 = mybir.dt.bfloat16
FP8 = mybir.dt.float8e4

N_CORES = 8
D = 1024          # model dim
ROWS = 8192       # B*T
M_CORE = ROWS // N_CORES   # rows per core
P = 128           # partitions
KT = D // P       # contraction tiles
MT = M_CORE // P  # row tiles per core
NCH = 2           # output-column chunks of 512
CHW = D // NCH    # 512
H = 16            # heads
DH = 64           # head dim
WSCALE = 64.0     # host pre-scale on Wq/Wk before fp8 quantization
INV_SQRT_DH = 1.0 / (DH ** 0.5)

_COMPILED = None
LAST_RESULT = None  # BassKernelResults of the most recent run (for test harness)


def _build():
    nc = bacc.Bacc("TRN2", target_bir_lowering=False, debug=False)

    hT_in = nc.dram_tensor("ht", [KT, P, M_CORE], BF16, kind="ExternalInput")
    h8_in = nc.dram_tensor("h8", [KT, P, M_CORE], FP8, kind="ExternalInput")
    w_in = {
        "wq": nc.dram_tensor("wq", [KT, P, D], FP8, kind="ExternalInput"),
        "wk": nc.dram_tensor("wk", [KT, P, D], FP8, kind="ExternalInput"),
        "wv": nc.dram_tensor("wv", [KT, P, D], BF16, kind="ExternalInput"),
        "wo": nc.dram_tensor("wo", [KT, P, D], BF16, kind="ExternalInput"),
    }
    out = nc.dram_tensor("out", [M_CORE, D], F32, kind="ExternalOutput")

    with tile.TileContext(nc) as tc:
        with (
            tc.tile_pool(name="res", bufs=1) as res_pool,
            tc.tile_pool(name="qsb", bufs=4) as q_pool,
            tc.tile_pool(name="sp", bufs=2) as sp_pool,
            tc.tile_pool(name="small", bufs=4) as small_pool,
            tc.tile_pool(name="y", bufs=2) as y_pool,
            tc.tile_pool(name="yT", bufs=MT) as yT_pool,
            tc.tile_pool(name="osb", bufs=3) as o_pool,
            tc.tile_pool(name="ps", bufs=1, space="PSUM") as ps_pool,
        ):
            # ---- resident operands, host-prepped layouts ----
            wsb = {
                name: res_pool.tile([P, KT, D], FP8 if name in ("wq", "wk")
                                    else BF16, tag=f"w_{name}", name=f"w_{name}")
                for name in ("wq", "wk", "wv", "wo")
            }
            hsb = res_pool.tile([P, KT, M_CORE], BF16, tag="h", name="h")
            h8sb = res_pool.tile([P, KT, M_CORE], FP8, tag="h8", name="h8")

            def tr(ap):
                return ap.rearrange("a p m -> p a m")

            A_COLS = 4 * P  # columns (rows of h) used by tiles 0-3

            # ---- Both HWDGE streams zig-zag in strict first-need order so
            # each phase-A round's (wq block, h8 block) pair arrives on two
            # different queues, then wk / wv / hbf-A / wo in deadline order.
            # Out-DMA triggers / yT transposes are appended later per tile.
            nc.scalar.dma_start(out=wsb["wq"][:, 0:2, :], in_=tr(w_in["wq"][0:2]))
            nc.sync.dma_start(out=h8sb[:, 0:2, 0:A_COLS],
                              in_=tr(h8_in[0:2, :, 0:A_COLS]))
            nc.sync.dma_start(out=wsb["wq"][:, 2:4, :], in_=tr(w_in["wq"][2:4]))
            nc.scalar.dma_start(out=h8sb[:, 2:4, 0:A_COLS],
                                in_=tr(h8_in[2:4, :, 0:A_COLS]))
            nc.scalar.dma_start(out=wsb["wq"][:, 4:6, :], in_=tr(w_in["wq"][4:6]))
            nc.sync.dma_start(out=h8sb[:, 4:6, 0:A_COLS],
                              in_=tr(h8_in[4:6, :, 0:A_COLS]))
            nc.sync.dma_start(out=wsb["wq"][:, 6:8, :], in_=tr(w_in["wq"][6:8]))
            nc.scalar.dma_start(out=h8sb[:, 6:8, 0:A_COLS],
                                in_=tr(h8_in[6:8, :, 0:A_COLS]))
            nc.sync.dma_start(out=wsb["wk"][:, 0:4, :], in_=tr(w_in["wk"][0:4]))
            nc.scalar.dma_start(out=wsb["wk"][:, 4:8, :], in_=tr(w_in["wk"][4:8]))
            nc.scalar.dma_start(out=wsb["wv"][:, 0:4, :], in_=tr(w_in["wv"][0:4]))
            nc.sync.dma_start(out=wsb["wv"][:, 4:8, :], in_=tr(w_in["wv"][4:8]))
            nc.scalar.dma_start(out=hsb[:, 0:4, 0:A_COLS],
                                in_=tr(hT_in[0:4, :, 0:A_COLS]))
            nc.sync.dma_start(out=hsb[:, 4:8, 0:A_COLS],
                              in_=tr(hT_in[4:8, :, 0:A_COLS]))
            nc.scalar.dma_start(out=wsb["wo"][:, 0:4, :], in_=tr(w_in["wo"][0:4]))
            nc.sync.dma_start(out=wsb["wo"][:, 4:8, :], in_=tr(w_in["wo"][4:8]))

            # ---- gpsimd SWDGE: tiles 4-7 halves of h8/hbf (needed ~25us+,
            # SWDGE starts ~10us and delivers ~130GB/s) ----
            scratch = res_pool.tile([P, CHW], BF16, tag="warm", name="warm")
            nc.gpsimd.memset(scratch, 0.0)
            for a in range(0, KT, 4):
                nc.gpsimd.dma_start(out=h8sb[:, a:a + 4, A_COLS:],
                                    in_=tr(h8_in[a:a + 4, :, A_COLS:]))
            for a in range(0, KT, 4):
                nc.gpsimd.dma_start(out=hsb[:, a:a + 4, A_COLS:],
                                    in_=tr(hT_in[a:a + 4, :, A_COLS:]))

            # 8 PSUM banks as 4 pairs of [128, 512] f32 tiles.
            def ps_pair(j):
                return [ps_pool.tile([P, CHW], F32, tag=f"T{2 * j + jo}",
                                     name=f"T{2 * j + jo}")
                        for jo in range(NCH)]

            def jsl(jo):
                return slice(jo * CHW, (jo + 1) * CHW)

            def v_mm(ps_t, i, kt, jo):
                nc.tensor.matmul(
                    out=ps_t,
                    lhsT=hsb[:, kt, i * P:(i + 1) * P],
                    rhs=wsb["wv"][:, kt, jsl(jo)],
                    start=(kt == 0),
                    stop=(kt == KT - 1),
                )

            def qk_mm(ps_t, wname, i, kt2, jo):
                # fp8 DoubleRow: both operands carry 2 contraction tiles.
                nc.tensor.matmul(
                    out=ps_t,
                    lhsT=h8sb[:, kt2:kt2 + 2, i * P:(i + 1) * P],
                    rhs=wsb[wname][:, kt2:kt2 + 2, jsl(jo)],
                    start=(kt2 == 0),
                    stop=(kt2 == KT - 2),
                    perf_mode=mybir.MatmulPerfMode.DoubleRow,
                )

            def q_copies(qp):
                # stage q in SBUF (bf16), chunk 0 on ACT, chunk 1 on DVE so
                # the pair frees in ~0.5us instead of ~1us.
                qsb = []
                for jo in range(NCH):
                    t_ = q_pool.tile([P, CHW], BF16, tag=f"qsb{jo}",
                                     name=f"qsb{jo}")
                    if jo == 0:
                        nc.scalar.copy(out=t_, in_=qp[jo])
                    else:
                        nc.vector.tensor_copy(out=t_, in_=qp[jo])
                    qsb.append(t_)
                return qsb

            def sp_reduce_gate(qsb, kp):
                # s[m,h] = sum_{d in head} q*k ; g = sigmoid(s * scale)
                # (scale folds away the fp8 WSCALE^2).  DVE muls+reduce,
                # ACT sigmoid.
                sp = sp_pool.tile([P, D], BF16, tag="sp", name="sp")
                for jo in range(NCH):
                    nc.vector.tensor_mul(out=sp[:, jsl(jo)], in0=qsb[jo],
                                         in1=kp[jo])
                s = small_pool.tile([P, H], F32, tag="s", name="s")
                nc.vector.reduce_sum(
                    out=s,
                    in_=sp.rearrange("p (h d) -> p h d", d=DH),
                    axis=mybir.AxisListType.X,
                )
                g = small_pool.tile([P, H], F32, tag="g", name="g")
                nc.scalar.activation(
                    out=g, in_=s,
                    func=mybir.ActivationFunctionType.Sigmoid,
                    scale=INV_SQRT_DH / (WSCALE * WSCALE),
                )
                return g

            yT_tiles = []

            def y_transpose(g, vp):
                # y = g*v per head (DVE, bf16); yT via Sync DMA transpose.
                y = y_pool.tile([P, D], BF16, tag="y", name="y")
                for jo in range(NCH):
                    g_sl = g[:, jo * (H // NCH):(jo + 1) * (H // NCH)]
                    g_bc = bass.AP(
                        tensor=g_sl.tensor, offset=g_sl.offset,
                        ap=[*g_sl.ap, [0, DH]],
                    )
                    nc.vector.tensor_mul(
                        out=y[:, jsl(jo)].rearrange("p (h d) -> p h d", d=DH),
                        in0=vp[jo].rearrange("p (h d) -> p h d", d=DH),
                        in1=g_bc,
                    )
                yT = yT_pool.tile([P, KT, P], BF16, tag="yT", name="yT")
                nc.sync.dma_start_transpose(out=yT, in_=y)
                yT_tiles.append(yT)

            def out_gemm(i, op, chunk_serial=False):
                # out tile i = yT_i @ wo, accumulated over kt into op[0/1].
                if chunk_serial:
                    orders = [(kt, jo) for jo in range(NCH) for kt in range(KT)]
                else:
                    orders = [(kt, jo) for kt in range(KT) for jo in range(NCH)]
                for kt, jo in orders:
                    nc.tensor.matmul(
                        out=op[jo],
                        lhsT=yT_tiles[i][:, kt, :],
                        rhs=wsb["wo"][:, kt, jsl(jo)],
                        start=(kt == 0),
                        stop=(kt == KT - 1),
                    )

            def out_drain(i, op, nsplit=2):
                # PSUM -> SBUF copies split ACT/DVE; DMA halves on BOTH
                # HWDGE queues (ACT chunk -> scalar queue, DVE chunk ->
                # sync queue) so the final drain halves.
                ms = slice(i * P, (i + 1) * P)
                osb = o_pool.tile([P, D], F32, tag="osb", name="osb")
                w_ = D // nsplit
                per = CHW // w_  # pieces per 512-col PSUM chunk
                for jo in range(nsplit):
                    sl = slice(jo * w_, (jo + 1) * w_)
                    src = op[jo // per][:, (jo % per) * w_:(jo % per) * w_ + w_]
                    if jo % 2 == 0:
                        nc.scalar.copy(out=osb[:, sl], in_=src)
                        nc.scalar.dma_start(out=out[ms, sl], in_=osb[:, sl])
                    else:
                        nc.vector.tensor_copy(out=osb[:, sl], in_=src)
                        nc.sync.dma_start(out=out[ms, sl], in_=osb[:, sl])

            # ---- warmup: 8 dependency-light matmuls to run the HAM clock
            # ramp during the initial DMA wait (gated only on the gpsimd
            # memset at ~5.9us).  Writes pair 0, overwritten by phase A. ----
            warm_ps = ps_pair(0)
            for _ in range(8):
                nc.tensor.matmul(out=warm_ps[0], lhsT=scratch[:, 0:P],
                                 rhs=scratch, start=True, stop=True)

            # ---- phase A: q tiles 0-3, kt2-outer, jo-outer in round ----
            qA = [ps_pair(t) for t in range(4)]
            for kt2 in range(0, KT, 2):
                for jo in range(NCH):
                    for t in range(4):
                        qk_mm(qA[t][jo], "wq", t, kt2, jo)

            # q copies in tile order (ACT+DVE split); k_t round 0 waits
            # only on tile t's pair being freed.
            qsbA = [q_copies(qA[t]) for t in range(4)]

            # ---- phase B: k rounds r0-r1 across tiles (wk[0:4] only),
            # then per-tile r2-r3 + score chain ----
            kB = [ps_pair(t) for t in range(4)]
            for kt2 in (0, 2):
                for t in range(4):
                    for jo in range(NCH):
                        qk_mm(kB[t][jo], "wk", t, kt2, jo)
            gC = [None] * 4
            for t in range(4):
                for kt2 in (4, 6):
                    for jo in range(NCH):
                        qk_mm(kB[t][jo], "wk", t, kt2, jo)
                gC[t] = sp_reduce_gate(qsbA[t], kB[t])

            # ---- phase C: v tiles 0-3 + y/transpose chains ----
            for t in range(4):
                vp = ps_pair(t)
                for kt in range(KT):
                    for jo in range(NCH):
                        v_mm(vp[jo], t, kt, jo)
                y_transpose(gC[t], vp)

            # ---- phase D: tiles 4-7: q,k fp8 rounds; v; out-GEMM t-4.
            # PSUM roles pinned: q->pair0, k->pair1, v->pair2, o->pair3. ----
            for t in range(4, MT):
                qp, kp = ps_pair(0), ps_pair(1)
                for kt2 in range(0, KT, 2):
                    for ps_t, wname, jo in ((qp[0], "wq", 0), (qp[1], "wq", 1),
                                            (kp[0], "wk", 0), (kp[1], "wk", 1)):
                        qk_mm(ps_t, wname, t, kt2, jo)
                qsb_t = q_copies(qp)
                g_t = sp_reduce_gate(qsb_t, kp)
                vp = ps_pair(2)
                for kt in range(KT):
                    for jo in range(NCH):
                        v_mm(vp[jo], t, kt, jo)
                op = ps_pair(3)
                out_gemm(t - 4, op)
                y_transpose(g_t, vp)
                out_drain(t - 4, op)

            # ---- phase E: out-GEMMs 4-7; last tile chunk-serial with
            # quarter-width drain so only ~1us trails the final matmul ----
            for ti, i in enumerate(range(4, MT)):
                op = ps_pair(ti)
                last = (i == MT - 1)
                out_gemm(i, op, chunk_serial=last)
                out_drain(i, op, nsplit=4 if last else 2)

    nc.compile()
    return nc


def kernel(hidden_states, m_gate, alpha_scale, Wq, Wk, Wv, Wo, Wa, ba, mix_logit,
           **_unused):
    global _COMPILED, LAST_RESULT
    if _COMPILED is None:
        _COMPILED = _build()
    nc = _COMPILED

    bf16 = ml_dtypes.bfloat16
    fp8 = ml_dtypes.float8_e4m3  # IEEE-style: max 240, matches TRN FP8_EXP4
    h = np.asarray(hidden_states, dtype=np.float32).reshape(ROWS, D)

    def prep_w(w, dtype, scale=1.0):
        # W [j, d] -> W^T [kt, p, j]: wT[kt, p, j] = W[j, kt*128+p]
        wt = np.ascontiguousarray(np.asarray(w, dtype=np.float32).T * scale)
        return wt.reshape(KT, P, D).astype(dtype)

    wq = prep_w(Wq, fp8, WSCALE)
    wk = prep_w(Wk, fp8, WSCALE)
    wv = prep_w(Wv, bf16)
    wo = prep_w(Wo, bf16)

    in_maps = []
    for c in range(N_CORES):
        hc = h[c * M_CORE:(c + 1) * M_CORE]  # [M_CORE, D]
        # hT [kt, p, m] = h[m, kt*128+p]
        ht = np.ascontiguousarray(hc.T.reshape(KT, P, M_CORE))
        in_maps.append({
            "ht": ht.astype(bf16), "h8": ht.astype(fp8),
            "wq": wq, "wk": wk, "wv": wv, "wo": wo,
        })

    res = run_bass_kernel_spmd(nc, in_maps, core_ids=list(range(N_CORES)))
    LAST_RESULT = res
    out = np.concatenate([res.results[c]["out"] for c in range(N_CORES)], axis=0)
    B, T = 4, 2048
    return out.reshape(B, T, D)
